# revision 16
# baseline (speedup 1.0000x reference)
"""Trainium2 Bass kernel: DarkChannelLoss.

Computes -mean(dark_channel(x)) for x [32,3,512,512] f32, where
dark_channel = reflect-pad(7) -> min over channels -> 15x15 sliding-window
min (windows clipped at bottom/right, i.e. +inf padded by 14).

Sharding: pure data parallel over batch, 4 images per NeuronCore x 8 cores.
Each core computes per-partition partial sums of its dark-channel map; the
host combines them into the final scalar mean.

Per-core pipeline (shapes hardcoded), processing images in pairs (BI=2):
  load:   one SWDGE cast DMA per pair (f32->bf16), source rows on partitions
          (4 row-tiles of 128), [128, BI, C, 4, 512].
  pass 1: channel-min, reflect pads along W (ACT reversed copies), then
          sliding-min cascade along W (windows 2,4,8,15) with ops batched
          over (image, row-tile) via 4D access patterns.
  transpose: 128x128 blocks via TensorE (identity matmul) into PSUM grouped
          per (image, W-tile), evacuated in [128,512] chunks by ScalarE.
  pass 2: sliding-min cascade along H batched over (W-tile, image); the sum
          is folded in via tensor_scalar accum_out. The narrow last W-tile
          (14 cols) of all 4 images is packed into one tile at partition
          offsets {0,32,64,96} via ScalarE cross-quad copies.
  out:    [128, 1] f32 per-partition partial sums; host reduces.
"""

import numpy as np

try:
    import concourse.bass as bass
except ImportError:  # pragma: no cover
    import sys

    sys.path.insert(0, "/opt/trn_rl_repo")
    import concourse.bass as bass

import concourse.mybir as mybir
import concourse.bacc as bacc
from concourse.tile import TileContext
from concourse.bass_utils import run_bass_kernel_spmd

F32 = mybir.dt.float32
BF16 = mybir.dt.bfloat16
INF = float("inf")
MIN = mybir.AluOpType.min

B, C, H, W = 32, 3, 512, 512
WIN = 15
PAD = WIN // 2          # 7
HP = H + 2 * PAD        # 526 padded rows
WP = W + 2 * PAD        # 526 padded cols
N_CORES = 8
N_IMG = B // N_CORES    # 4 images per core
NT = H // 128           # 4 row tiles of source rows
PT = (WP + 127) // 128  # 5 W tiles
FREE = PT * 128         # 640
MF = 544                # m/cascade tile free width (col = src_w + 8)
DEN = B * HP * WP


def build_program(n_img=N_IMG, bi=2):
    assert n_img % bi == 0
    nb = n_img // bi
    nc = bacc.Bacc("TRN2", target_bir_lowering=False, debug=False)
    x = nc.dram_tensor("x", [n_img, C, H, W], F32, kind="ExternalInput")
    out = nc.dram_tensor("out", [128, 1], F32, kind="ExternalOutput")

    n_acc = nb + 1  # one accum column per pair + one for the packed last W-tile

    with TileContext(nc) as tc:
        from contextlib import ExitStack

        with ExitStack() as ctx:
            constp = ctx.enter_context(tc.tile_pool(name="const", bufs=1))
            chp = ctx.enter_context(tc.tile_pool(name="ch", bufs=2))
            tmpp = ctx.enter_context(tc.tile_pool(name="tmp", bufs=1))
            mp = ctx.enter_context(tc.tile_pool(name="m", bufs=2))
            cascp = ctx.enter_context(tc.tile_pool(name="casc", bufs=3))
            rmp = ctx.enter_context(tc.tile_pool(name="rm", bufs=2))
            tbp = ctx.enter_context(tc.tile_pool(name="tb", bufs=2))
            tb4p = ctx.enter_context(tc.tile_pool(name="tb4", bufs=2))
            dcp = ctx.enter_context(tc.tile_pool(name="dc", bufs=2))
            accp = ctx.enter_context(tc.tile_pool(name="acc", bufs=1))
            psp = ctx.enter_context(tc.tile_pool(name="ps", bufs=4, space="PSUM"))

            ident = constp.tile([128, 128], BF16, tag="ident")
            idt = constp.tile([128, 128], mybir.dt.int16, tag="idt")
            nc.gpsimd.iota(idt[:, :], pattern=[[1, 128]], base=0, channel_multiplier=-1)
            nc.vector.tensor_single_scalar(
                ident[:, :], idt[:, :], 0, mybir.AluOpType.is_equal
            )
            acc = accp.tile([128, n_acc], F32, tag="acc")
            nc.vector.memset(acc[:, :], 0.0)
            # packed tile for the narrow last W-tile of all n_img images:
            # image i sits at partitions 32i..32i+14; other lanes stay 0.0
            tbP4 = accp.tile([128, FREE], BF16, tag="tbP4")
            nc.gpsimd.memset(tbP4[:, :], 0.0)

            for b in range(nb):
                # ---- load: one cast DMA for the image pair ----
                cht = chp.tile([128, bi, C, NT, W], BF16, tag="ch", name=f"ch_{b}")
                nc.gpsimd.dma_start(
                    cht[:, :, :, :, :],
                    x[bi * b : bi * (b + 1)].rearrange(
                        "b c (q p) w -> p b c q w", p=128
                    ),
                )

                # ---- pass 1: channel-min + W cascade, merged over (img, t) ----
                # m column layout: col = src_w + 8, so padded col j lives at j+1
                tmp = tmpp.tile([128, bi, NT, W], BF16, tag="tmp", name=f"tmp_{b}")
                nc.vector.tensor_tensor(
                    tmp[:, :, :, :], cht[:, :, 0], cht[:, :, 1], MIN
                )
                m = mp.tile([128, bi, NT, MF], BF16, tag="m", name=f"m_{b}")
                nc.gpsimd.memset(m[:, :, :, 8 + W : MF], INF)
                nc.vector.tensor_tensor(
                    m[:, :, :, 8 : 8 + W], tmp[:, :, :, :], cht[:, :, 2], MIN
                )
                # reflect pads along W: padded 0..6 <- src 7..1 (cols 15..9),
                # padded 519..525 <- src 510..504 (cols 518..512)
                nc.scalar.copy(m[:, :, :, 1:8], m[:, :, :, 15:8:-1])
                nc.scalar.copy(m[:, :, :, 520:527], m[:, :, :, 518:511:-1])

                w2 = cascp.tile([128, bi, NT, MF], BF16, tag="casc", name=f"w2_{b}")
                w4 = cascp.tile([128, bi, NT, MF], BF16, tag="casc", name=f"w4_{b}")
                w8 = cascp.tile([128, bi, NT, MF], BF16, tag="casc", name=f"w8_{b}")
                # w2 col j = min(padded j, j+1) = min(m[j+1], m[j+2])
                n2 = WP + WIN - 2  # 539
                nc.vector.tensor_tensor(
                    w2[:, :, :, 0:n2], m[:, :, :, 1 : n2 + 1], m[:, :, :, 2 : n2 + 2], MIN
                )
                n4 = n2 - 2
                nc.vector.tensor_tensor(
                    w4[:, :, :, 0:n4], w2[:, :, :, 0:n4], w2[:, :, :, 2 : n4 + 2], MIN
                )
                n8 = n4 - 4
                nc.vector.tensor_tensor(
                    w8[:, :, :, 0:n8], w4[:, :, :, 0:n8], w4[:, :, :, 4 : n8 + 4], MIN
                )
                rm = rmp.tile([128, bi, NT, FREE], BF16, tag="rm", name=f"rm_{b}")
                nc.gpsimd.memset(rm[:, :, :, WP:FREE], INF)
                nc.vector.tensor_tensor(
                    rm[:, :, :, 0:512], w8[:, :, :, 0:512], w8[:, :, :, PAD : 512 + PAD], MIN
                )
                nc.vector.tensor_tensor(
                    rm[:, :, :, 512:WP], w8[:, :, :, 512:WP], w8[:, :, :, 512 + PAD : WP + PAD], MIN
                )

                # ---- transpose: TensorE -> PSUM (per image, W-tile), ACT evac ----
                tbm = tbp.tile([128, PT - 1, bi, FREE], BF16, tag="tb", name=f"tb_{b}")
                tb4 = tb4p.tile([128, bi, FREE], BF16, tag="tb4", name=f"tb4_{b}")
                nc.gpsimd.memset(tbm[:, :, :, WP:FREE], INF)
                nc.gpsimd.memset(tb4[:, :, WP:FREE], INF)
                for ii in range(bi):
                    for p in range(PT):
                        pst = psp.tile(
                            [128, NT, 128], BF16, tag="pst", name=f"pst_{b}_{ii}_{p}"
                        )
                        for t in range(NT):
                            nc.tensor.transpose(
                                pst[:, t, :],
                                rm[:, ii, t, 128 * p : 128 * (p + 1)],
                                ident[:, :],
                            )
                        if p < PT - 1:
                            nc.scalar.copy(tbm[:, p, ii, PAD : PAD + H], pst[:, :, :])
                        else:
                            nc.scalar.copy(tb4[:, ii, PAD : PAD + H], pst[:, :, :])

                # row reflection on the free dim (padded rows 0..6 <- 14..8,
                # 519..525 <- 517..511) for the main tiles and the narrow tile
                nc.scalar.copy(tbm[:, :, :, 0:PAD], tbm[:, :, :, 2 * PAD : PAD : -1])
                nc.scalar.copy(
                    tbm[:, :, :, H + PAD : HP], tbm[:, :, :, H + PAD - 2 : H - 2 : -1]
                )
                nc.scalar.copy(tb4[:, :, 0:PAD], tb4[:, :, 2 * PAD : PAD : -1])
                nc.scalar.copy(
                    tb4[:, :, H + PAD : HP], tb4[:, :, H + PAD - 2 : H - 2 : -1]
                )
                # pack valid 14 partitions of each image into tbP4 quad slots
                for ii in range(bi):
                    i = bi * b + ii
                    nc.scalar.copy(tbP4[32 * i : 32 * i + 14, :], tb4[0:14, ii, :])

                # ---- pass 2: H cascade + accumulate (merged over (p, img)) ----
                h2 = cascp.tile([128, PT - 1, bi, MF], BF16, tag="casc", name=f"h2_{b}")
                h4 = cascp.tile([128, PT - 1, bi, MF], BF16, tag="casc", name=f"h4_{b}")
                h8 = cascp.tile([128, PT - 1, bi, MF], BF16, tag="casc", name=f"h8_{b}")
                n2 = HP + WIN - 2
                nc.vector.tensor_tensor(
                    h2[:, :, :, 0:n2], tbm[:, :, :, 0:n2], tbm[:, :, :, 1 : n2 + 1], MIN
                )
                n4 = n2 - 2
                nc.vector.tensor_tensor(
                    h4[:, :, :, 0:n4], h2[:, :, :, 0:n4], h2[:, :, :, 2 : n4 + 2], MIN
                )
                n8 = n4 - 4
                nc.vector.tensor_tensor(
                    h8[:, :, :, 0:n8], h4[:, :, :, 0:n8], h4[:, :, :, 4 : n8 + 4], MIN
                )
                dc = dcp.tile([128, PT - 1, bi, HP], BF16, tag="dc", name=f"dc_{b}")
                nc.vector.tensor_tensor(
                    dc[:, :, :, :], h8[:, :, :, 0:HP], h8[:, :, :, PAD : HP + PAD], MIN
                )
                # free-dim sum via tensor_scalar accumulator (in-place bypass)
                nc.vector.tensor_scalar(
                    dc[:, :, :, :],
                    dc[:, :, :, :],
                    0.0,
                    0.0,
                    mybir.AluOpType.bypass,
                    mybir.AluOpType.add,
                    accum_out=acc[:, b : b + 1],
                )

            # ---- packed last W-tile: one cascade for all images ----
            g2 = cascp.tile([128, MF], BF16, tag="casc", name="g2")
            g4 = cascp.tile([128, MF], BF16, tag="casc", name="g4")
            g8 = cascp.tile([128, MF], BF16, tag="casc", name="g8")
            n2 = HP + WIN - 2
            nc.vector.tensor_tensor(g2[:, 0:n2], tbP4[:, 0:n2], tbP4[:, 1 : n2 + 1], MIN)
            n4 = n2 - 2
            nc.vector.tensor_tensor(g4[:, 0:n4], g2[:, 0:n4], g2[:, 2 : n4 + 2], MIN)
            n8 = n4 - 4
            nc.vector.tensor_tensor(g8[:, 0:n8], g4[:, 0:n8], g4[:, 4 : n8 + 4], MIN)
            gdc = dcp.tile([128, HP], BF16, tag="gdc", name="gdc")
            nc.vector.tensor_tensor(gdc[:, :], g8[:, 0:HP], g8[:, PAD : HP + PAD], MIN)
            nc.vector.tensor_scalar(
                gdc[:, :],
                gdc[:, :],
                0.0,
                0.0,
                mybir.AluOpType.bypass,
                mybir.AluOpType.add,
                accum_out=acc[:, nb : nb + 1],
            )

            tot = accp.tile([128, 1], F32, tag="tot")
            nc.vector.tensor_reduce(
                tot[:, 0:1],
                acc[:, 0:n_acc],
                axis=mybir.AxisListType.X,
                op=mybir.AluOpType.add,
            )
            nc.sync.dma_start(out[:, :], tot[:, :])

    return nc


_PROGRAM = None


def _get_program():
    global _PROGRAM
    if _PROGRAM is None:
        _PROGRAM = build_program()
        _PROGRAM.finalize()  # run Bacc passes (wait splitting, regalloc)
    return _PROGRAM


def kernel(generated_image):
    x = np.ascontiguousarray(np.asarray(generated_image), dtype=np.float32)
    assert x.shape == (B, C, H, W)
    nc = _get_program()
    shards = x.reshape(N_CORES, N_IMG, C, H, W)
    in_maps = [{"x": np.ascontiguousarray(shards[i])} for i in range(N_CORES)]
    res = run_bass_kernel_spmd(nc, in_maps, list(range(N_CORES)))
    total = float(np.sum([r["out"].astype(np.float64).sum() for r in res.results]))
    return np.array(-total / DEN, dtype=np.float32)
